# revision 2
# baseline (speedup 1.0000x reference)
"""Trainium2 Bass kernel for nn_BinaryMemory (retrieval_knn).

reference:
    gated = sigmoid(query @ W.T + b)                      # [1, D], D=4096
    sims  = 1 - mean(|memory - gated|, axis=-1)           # [N],   N=16384
    mask  = sims >= 0.8

Sharding (8 cores, no collectives): shard the D axis. Core c owns
d-chunk [c*512, (c+1)*512):
  - W rows c*512..c*512+511  -> computes gated[c*512:(c+1)*512] locally
    (dot products via fused tensor_tensor_reduce on DVE)
  - memory[:, c*512:(c+1)*512] -> partial L1 sums over its d-chunk for
    all 16384 rows (DVE subtract + ScalarE Abs-with-accumulate)
  - outputs partials [16384]; host sums the 8 partials and applies
    sims = 1 - s/D, mask = sims >= 0.8.

Per-core HBM traffic ~42 MB (memory regime); DVE ~90us and ACT ~80us hide
under the ~117us DMA roofline.
"""
import sys

sys.path.insert(0, "/opt/trn_rl_repo")

import numpy as np

import concourse.bacc as bacc
import concourse.mybir as mybir
import concourse.tile as tile
from concourse.bass_utils import run_bass_kernel_spmd

N_CORES = 8
D = 4096
N = 16384
D_SH = D // N_CORES          # 512 dims per core
W_TILES = D_SH // 128        # 4   gate-weight tiles [128, 4096]
GROUPS_PER_TILE = 8          # row-groups packed per memory tile
M_TILES = N // (128 * GROUPS_PER_TILE)   # 16 memory tiles [128, 8*512]
THRESHOLD = 0.8

_CACHE = {}


def _build():
    f32 = mybir.dt.float32
    nc = bacc.Bacc(
        "TRN2", target_bir_lowering=False, debug=False, num_devices=N_CORES
    )

    query = nc.dram_tensor("query", [1, D], f32, kind="ExternalInput")
    w = nc.dram_tensor("w", [D_SH, D], f32, kind="ExternalInput")
    b = nc.dram_tensor("b", [D_SH], f32, kind="ExternalInput")
    mem = nc.dram_tensor("mem", [N, D_SH], f32, kind="ExternalInput")
    ident = nc.dram_tensor("ident", [128, 128], f32, kind="ExternalInput")
    partials = nc.dram_tensor("partials", [N], f32, kind="ExternalOutput")

    with tile.TileContext(nc) as tc:
        with (
            tc.tile_pool(name="const", bufs=1) as cpool,
            tc.tile_pool(name="wq", bufs=2) as wpool,
            tc.tile_pool(name="mem", bufs=5) as mpool,
            tc.tile_pool(name="diff", bufs=2) as dpool,
            tc.tile_pool(name="absout", bufs=2) as apool,
            tc.tile_pool(name="small", bufs=1) as spool,
            tc.tile_pool(name="psum", bufs=2, space="PSUM") as ppool,
            tc.tile_pool(name="dram", bufs=1, space="DRAM") as drpool,
        ):
            id_sb = cpool.tile([128, 128], f32, tag="ident")
            nc.sync.dma_start(out=id_sb[:], in_=ident[:])

            # ---- gate: z[j] = sum_d W[j, d] * q[d], j = wt*128 + p ----
            q_b = cpool.tile([128, D], f32, tag="qb")
            nc.sync.dma_start(out=q_b[:], in_=query[:].to_broadcast((128, D)))

            z_col = spool.tile([128, W_TILES], f32, tag="zcol")
            for wt in range(W_TILES):
                w_tile = wpool.tile([128, D], f32, tag="wt")
                nc.sync.dma_start(
                    out=w_tile[:], in_=w[wt * 128 : (wt + 1) * 128, :]
                )
                scratch = dpool.tile([128, D], f32, tag="diff")
                nc.vector.scalar_tensor_tensor(
                    out=scratch[:],
                    in0=w_tile[:],
                    scalar=1.0,
                    in1=q_b[:],
                    op0=mybir.AluOpType.mult,
                    op1=mybir.AluOpType.mult,
                    accum_out=z_col[:, wt : wt + 1],
                )

            # z + b, sigmoid (still column layout: [p, wt] = j = wt*128+p)
            b_col = spool.tile([128, W_TILES], f32, tag="bcol")
            nc.sync.dma_start(
                out=b_col[:], in_=b[:].rearrange("(t p) -> p t", p=128)
            )
            g_col = spool.tile([128, W_TILES], f32, tag="gcol")
            nc.vector.tensor_add(g_col[:], z_col[:], b_col[:])
            nc.scalar.activation(
                g_col[:], g_col[:], mybir.ActivationFunctionType.Sigmoid
            )

            # transpose to row layout and bounce through DRAM to broadcast
            g_ps = ppool.tile([W_TILES, 128], f32, tag="gps")
            nc.tensor.transpose(g_ps[:], g_col[:], id_sb[:])
            g_row = spool.tile([W_TILES, 128], f32, tag="grow")
            nc.vector.tensor_copy(g_row[:], g_ps[:])
            g_dram = drpool.tile([D_SH], f32, tag="gdram")
            nc.sync.dma_start(
                out=g_dram[:].rearrange("(t p) -> t p", p=128), in_=g_row[:]
            )
            # G_rep[p, j, d] = gated[d] for every partition p and group j
            g_rep = cpool.tile([128, GROUPS_PER_TILE * D_SH], f32, tag="grep")
            nc.sync.dma_start(
                out=g_rep[:].rearrange("p (j d) -> p j d", j=GROUPS_PER_TILE),
                in_=g_dram[:]
                .unsqueeze(0)
                .unsqueeze(0)
                .to_broadcast((128, GROUPS_PER_TILE, D_SH)),
            )

            # ---- sims partials ----
            # memory tile t: partition p, free (j, d) = mem[(t*8+j)*128 + p, d]
            memv = mem[:].rearrange(
                "(t j p) d -> t p j d", p=128, j=GROUPS_PER_TILE
            )
            sums = spool.tile([128, M_TILES * GROUPS_PER_TILE], f32, tag="sums")
            for t in range(M_TILES):
                m_tile = mpool.tile([128, GROUPS_PER_TILE * D_SH], f32, tag="m")
                nc.sync.dma_start(
                    out=m_tile[:].rearrange(
                        "p (j d) -> p j d", j=GROUPS_PER_TILE
                    ),
                    in_=memv[t],
                )
                diff = dpool.tile([128, GROUPS_PER_TILE * D_SH], f32, tag="diff")
                nc.vector.tensor_sub(diff[:], m_tile[:], g_rep[:])
                for j in range(GROUPS_PER_TILE):
                    a_out = apool.tile([128, D_SH], f32, tag="absout")
                    col = t * GROUPS_PER_TILE + j
                    nc.scalar.activation(
                        a_out[:],
                        diff[:, j * D_SH : (j + 1) * D_SH],
                        mybir.ActivationFunctionType.Abs,
                        accum_out=sums[:, col : col + 1],
                    )

            # transpose [p, group] -> [group, p] so DRAM write is contiguous
            s_ps = ppool.tile([128, 128], f32, tag="sps")
            nc.tensor.transpose(s_ps[:], sums[:], id_sb[:])
            s_out = spool.tile([128, 128], f32, tag="sout")
            nc.vector.tensor_copy(s_out[:], s_ps[:])
            nc.sync.dma_start(
                out=partials[:].rearrange("(g p) -> g p", p=128), in_=s_out[:]
            )

    nc.compile()
    return nc


def _get_nc():
    if "nc" not in _CACHE:
        _CACHE["nc"] = _build()
    return _CACHE["nc"]


def kernel(query, W, b, memory, _trace=False, _return_raw=False):
    query = np.ascontiguousarray(np.asarray(query, dtype=np.float32))
    W = np.asarray(W, dtype=np.float32)
    b = np.asarray(b, dtype=np.float32)
    memory = np.asarray(memory, dtype=np.float32)
    ident = np.eye(128, dtype=np.float32)

    in_maps = []
    for c in range(N_CORES):
        sl = slice(c * D_SH, (c + 1) * D_SH)
        in_maps.append(
            {
                "query": query,
                "w": np.ascontiguousarray(W[sl, :]),
                "b": np.ascontiguousarray(b[sl]),
                "mem": np.ascontiguousarray(memory[:, sl]),
                "ident": ident,
            }
        )

    nc = _get_nc()
    res = run_bass_kernel_spmd(
        nc, in_maps, list(range(N_CORES)), trace=_trace
    )

    total = np.zeros(N, dtype=np.float64)
    for c in range(N_CORES):
        total += res.results[c]["partials"].astype(np.float64)
    sims = (1.0 - total / D).astype(np.float32)
    mask = sims >= THRESHOLD
    if _return_raw:
        return (sims, mask), res
    return sims, mask
